# revision 3
# baseline (speedup 1.0000x reference)
"""TRN2 Bass kernel for nn_CNNDSTv2_batch: out = mobius16(zeta16(M[:,0]) * zeta16(M[:,1])).

Math: the 16-bit superset-zeta factorizes as Z = A8 @ X @ A8^T on the 256x256
view X[hi_byte, lo_byte]; A8 = [[A7, A7], [0, A7]] block-triangular, so each
8-bit stage is 3 accumulating 128x128 matmuls reusing one stationary. Each
two-sided transform runs as [stage, PE-transpose, stage] and yields the
transposed result; chaining zeta -> multiply -> mobius lands back in natural
layout. Matmuls run in f32r (tf32-like, 11 explicit mantissa bits, 1 cyc/row):
stage inputs are 2-term f32r hi/lo splits (~23-bit precision, exact for our
0/+-1 stationaries) except the raw input stage where a single rounding is
consistency-safe (commutes with the positive-sum conjunction).

Sharding: pure data parallel, batch 512 -> 64 per core across 8 cores.
"""
import sys
import os
import functools

sys.path.insert(0, "/opt/trn_rl_repo")
import numpy as np

BATCH = 512
L = 65536
NCORES = 8
BPC = BATCH // NCORES          # 64 batch elems per core
PAIRS = BPC // 2               # 2 elems per pipeline iteration


def _pc(v):
    return bin(v).count("1")


def _constants():
    k = np.arange(128)
    sup = (k[:, None] & k[None, :]) == k[None, :]          # sup[k,m] = k superset of m
    AT7 = sup.astype(np.float32)                           # lhsT for A7 @ x
    pc = np.array([_pc(i) for i in range(128)])
    sign = (-1.0) ** (pc[:, None] - pc[None, :])
    BT7 = (sup * sign).astype(np.float32)                  # lhsT for B7 @ x
    return AT7, BT7


def _build():
    import concourse.bacc as bacc
    import concourse.tile as tile
    import concourse.mybir as mybir

    dt = mybir.dt
    F32, F32R = dt.float32, dt.float32r

    nc = bacc.Bacc("TRN2", target_bir_lowering=False, debug=False)

    # HBM layout (host pre-permuted, all DMAs contiguous):
    # Mi[pair, ch, p(=bits14..8), (b, I=bit15, J=bit7, l=bits6..0)]
    Mi = nc.dram_tensor("Mi", [PAIRS, 2, 128, 1024], F32R, kind="ExternalInput").ap()
    # C = [AT7 | BT7 | -BT7] as f32r (exact 0/+-1), Id = f32 identity
    C = nc.dram_tensor("C", [128, 384], F32R, kind="ExternalInput").ap()
    Id_d = nc.dram_tensor("Id", [128, 128], F32, kind="ExternalInput").ap()
    # O[pair, I''(=bit15), p(=bits14..8), (b, J=bit7, l=bits6..0)]
    O = nc.dram_tensor("O", [PAIRS, 2, 128, 512], F32, kind="ExternalOutput").ap()

    with tile.TileContext(nc) as tc:
        with tc.tile_pool(name="const", bufs=1) as cp, \
             tc.tile_pool(name="sbuf", bufs=2) as sb, \
             tc.tile_pool(name="psum", bufs=8, space="PSUM") as ps:
            Ct = cp.tile([128, 384], F32R, tag="C")
            nc.sync.dma_start(Ct[:], C)
            Id = cp.tile([128, 128], F32, tag="Id")
            nc.sync.dma_start(Id[:], Id_d)
            AT = Ct[:, 0:128]
            BT = Ct[:, 128:256]
            nBT = Ct[:, 256:384]

            def mm(out_ap, lhsT, rhs, start, stop):
                nc.tensor.matmul(out_ap, lhsT, rhs, start=start, stop=stop)

            def stage(dst0, dst1, M, Mn, s0, s1):
                """dst0 = M@s0h + M@s0l + Mn@s1h + Mn@s1l ; dst1 = M@s1h + M@s1l.
                s0/s1 are lists of moving APs (1 or 2 planes)."""
                n = len(s0)
                for i, a in enumerate(s0):
                    mm(dst0, M, a, start=(i == 0), stop=False)
                for i, a in enumerate(s1):
                    mm(dst0, Mn, a, start=False, stop=(i == n - 1))
                for i, a in enumerate(s1):
                    mm(dst1, M, a, start=(i == 0), stop=(i == n - 1))

            def transpose4(dst, src0, src1):
                """8 [128,128] PE transposes assembling dst[J][:, (b, K, :)] from
                src[K][:, (b, J, :)] for J,K,b in {0,1}; dst/src = 2 tiles [128,512]."""
                for Jd, d in enumerate(dst):
                    k = 0
                    for b in (0, 1):
                        for K, s in enumerate((src0, src1)):
                            nc.tensor.matmul(
                                d[:, (b * 256 + K * 128):(b * 256 + K * 128 + 128)],
                                s[:, (b * 256 + Jd * 128):(b * 256 + Jd * 128 + 128)],
                                Id[:], is_transpose=True,
                                start=(k == 0), stop=(k == 3))
                            k += 1

            def split(hi_t, lo_t, src_psum):
                """hi = round_f32r(src), lo = round_f32r(src - hi); feed as 2 planes."""
                nc.scalar.copy(hi_t[:], src_psum[:])
                nc.vector.tensor_sub(lo_t[:], src_psum[:], hi_t[:].bitcast(F32))

            for pr in range(PAIRS):
                z0s = [None, None]
                qh = [None, None]
                ql = [None, None]
                for c in (0, 1):
                    xin = sb.tile([128, 1024], F32R, tag=f"xin{c}")
                    nc.sync.dma_start(xin[:], Mi[pr, c])
                    xr = xin[:].rearrange("p (b i f) -> p b i f", b=2, i=2)
                    xI = [xr[:, :, 0], xr[:, :, 1]]

                    # zeta stage 1 (hi group: bit15 blocks + bits14..8 matmul)
                    y0 = ps.tile([128, 512], F32, tag="ps")
                    y1 = ps.tile([128, 512], F32, tag="ps")
                    stage(y0[:], y1[:], AT, AT, [xI[0]], [xI[1]])
                    ys0 = sb.tile([128, 512], F32, tag=f"ys0_{c}")
                    ys1 = sb.tile([128, 512], F32, tag=f"ys1_{c}")
                    nc.scalar.copy(ys0[:], y0[:])
                    nc.scalar.copy(ys1[:], y1[:])

                    # transpose: yt[J][p=l7, f=(b, I', h7')]
                    yt0 = ps.tile([128, 512], F32, tag="ps")
                    yt1 = ps.tile([128, 512], F32, tag="ps")
                    transpose4((yt0, yt1), ys0, ys1)

                    # split to f32r hi/lo
                    yth0 = sb.tile([128, 512], F32R, tag=f"yth0_{c}")
                    ytl0 = sb.tile([128, 512], F32R, tag=f"ytl0_{c}")
                    yth1 = sb.tile([128, 512], F32R, tag=f"yth1_{c}")
                    ytl1 = sb.tile([128, 512], F32R, tag=f"ytl1_{c}")
                    split(yth0, ytl0, yt0)
                    split(yth1, ytl1, yt1)

                    # zeta stage 2 (lo group: bit7 blocks + bits6..0 matmul)
                    z0 = ps.tile([128, 512], F32, tag="ps")
                    z1 = ps.tile([128, 512], F32, tag="ps")
                    stage(z0[:], z1[:], AT, AT,
                          [yth0[:], ytl0[:]], [yth1[:], ytl1[:]])

                    if c == 0:
                        for Jp, z in enumerate((z0, z1)):
                            z0s[Jp] = sb.tile([128, 512], F32, tag=f"z0s{Jp}", name=f"z0s{Jp}")
                            nc.vector.tensor_copy(z0s[Jp][:], z[:])
                    else:
                        for Jp, z in enumerate((z0, z1)):
                            t = sb.tile([128, 512], F32, tag=f"t{Jp}")
                            nc.vector.tensor_mul(t[:], z[:], z0s[Jp][:])
                            qh[Jp] = sb.tile([128, 512], F32R, tag=f"qh{Jp}", name=f"qh{Jp}")
                            nc.vector.tensor_copy(qh[Jp][:], t[:])
                            ql[Jp] = sb.tile([128, 512], F32R, tag=f"ql{Jp}", name=f"ql{Jp}")
                            nc.vector.tensor_sub(ql[Jp][:], t[:], qh[Jp][:].bitcast(F32))

                # mobius stage 1 (lo' group)
                u0 = ps.tile([128, 512], F32, tag="ps")
                u1 = ps.tile([128, 512], F32, tag="ps")
                stage(u0[:], u1[:], BT, nBT,
                      [qh[0][:], ql[0][:]], [qh[1][:], ql[1][:]])
                us0 = sb.tile([128, 512], F32, tag="us0")
                us1 = sb.tile([128, 512], F32, tag="us1")
                nc.scalar.copy(us0[:], u0[:])
                nc.scalar.copy(us1[:], u1[:])

                ut0 = ps.tile([128, 512], F32, tag="ps")
                ut1 = ps.tile([128, 512], F32, tag="ps")
                transpose4((ut0, ut1), us0, us1)

                uth0 = sb.tile([128, 512], F32R, tag="uth0")
                utl0 = sb.tile([128, 512], F32R, tag="utl0")
                uth1 = sb.tile([128, 512], F32R, tag="uth1")
                utl1 = sb.tile([128, 512], F32R, tag="utl1")
                split(uth0, utl0, ut0)
                split(uth1, utl1, ut1)

                # mobius stage 2 (hi' group)
                o0 = ps.tile([128, 512], F32, tag="ps")
                o1 = ps.tile([128, 512], F32, tag="ps")
                stage(o0[:], o1[:], BT, nBT,
                      [uth0[:], utl0[:]], [uth1[:], utl1[:]])
                for Ipp, o in enumerate((o0, o1)):
                    osb = sb.tile([128, 512], F32, tag=f"os{Ipp}")
                    nc.scalar.copy(osb[:], o[:])
                    nc.sync.dma_start(O[pr, Ipp], osb[:])

    nc.compile()
    return nc


@functools.lru_cache(maxsize=1)
def _get_nc():
    return _build()


def _host_in(M):
    """M [512, 2, 65536] f32 -> per-core Mi [PAIRS, 2, 128, 1024] contiguous.
    index16 = I*2^15 + p*2^8 + J*2^7 + l ; f-order (b, I, J, l)."""
    M6 = np.asarray(M, dtype=np.float32).reshape(NCORES, PAIRS, 2, 2, 2, 128, 2, 128)
    #                                      core, pair, b,  ch, I,  p,   J,  l
    Mi = np.ascontiguousarray(M6.transpose(0, 1, 3, 5, 2, 4, 6, 7))
    #                                      core, pair, ch, p, b, I, J, l
    return Mi.reshape(NCORES, PAIRS, 2, 128, 1024)


def _host_out(Os):
    """Os list of [PAIRS, 2, 128, 512] per core -> [512, 65536, 1, 1]."""
    O = np.stack(Os).reshape(NCORES, PAIRS, 2, 128, 2, 2, 128)
    #                         core, pair, I, p, b, J, l
    out = np.ascontiguousarray(O.transpose(0, 1, 4, 2, 3, 5, 6))
    #                                      core, pair, b, I, p, J, l
    return out.reshape(BATCH, L, 1, 1)


def _run(M, trace=False):
    from concourse.bass_utils import run_bass_kernel_spmd
    nc = _get_nc()
    AT7, BT7 = _constants()
    C = np.concatenate([AT7, BT7, -BT7], axis=1)
    Id = np.eye(128, dtype=np.float32)
    Mi = _host_in(M)
    in_maps = [{"Mi": Mi[k], "C": C, "Id": Id} for k in range(NCORES)]
    res = run_bass_kernel_spmd(nc, in_maps, list(range(NCORES)), trace=trace)
    out = _host_out([res.results[k]["O"] for k in range(NCORES)])
    return out, res


def kernel(M):
    out, _ = _run(M, trace=False)
    return out
